# revision 23
# baseline (speedup 1.0000x reference)
"""Trainium2 Bass kernel for nn_DAMIC_88235808129614.

TextCNN (embed -> conv fs=3/4/5 -> relu -> maxpool) + 2-layer LSTM + sigmoid
head. Data-parallel over batch across 8 NeuronCores; the sequential timestep
loop runs locally per shard (no collectives).

LSTM gate matmuls use 4x PE column tiling (128x32 mode): the batch dim is
only 8, so 4 independent gate-quarter streams share the array, each writing
its own PSUM partition group (one gate = 4 concurrent 256-col weight
streams). All recurrent weights are SBUF-resident: whh0 in fp16, wih1/whh1
in fp8e4 (all gate-weight inputs pre-scaled x16 so fp8 values sit in the
e4m3 normal range; the gate activations apply scale=1/16). Per step, the
weight streams that depend only on last-step state (whh0, whh1) issue
before those needing fresh state (prevT, wih1), hiding the cell-update and
head latency under PE streaming. h transposes are row-tiled PE transposes,
one PSUM bank per row group (concurrent row tiles must not share a bank).
The CNN embeds via a casting indirect gather (f16 table -> f32 rows) and
per-128-token PE transposes; conv runs as E-chunked matmuls with
relu+maxpool fused on ACT/DVE.

kernel(**inputs) takes the FULL unsharded inputs and returns [64, 50, 32] f32.
"""
import numpy as np

import concourse.bass as bass
import concourse.mybir as mybir
import concourse.tile as tile
from concourse.bass_utils import run_bass_kernel_spmd
from concourse.masks import make_identity


def _patched_drain_and_barrier(self, tick_clock, wait_clock):
    drain_inst = self.nc.sync.drain()
    wait_clock.add_sem_waits(
        drain_inst.ins, tile.ScopedClock({None: tick_clock.global_clock})
    )
    si = drain_inst.ins.sync_info
    waits = list(si.on_wait) if si and si.on_wait else []
    if len(waits) > 1:
        si.on_wait = waits[:1]
        for w in waits[1:]:
            nop = self.nc.sync.nop(nofuse=True, hint="split_drain_wait")
            nsi = nop.ins.sync_info
            if nsi is None:
                nop.ins.sync_info = mybir.SyncInfo(on_wait=[w], on_update=[])
            else:
                nsi.on_wait = [w]
    self.nc.all_engine_barrier()
    assert self.sems is not None
    popped = self.nc._tile_sem_poison_stack.pop()
    assert popped is self._sem_poison
    self.nc.clear_and_free_semaphores(list(self.sems.allocated().values()))
    self.nc.all_engine_barrier()


def split_multiwait(nc, max_waits=1):
    """This walrus build rejects instructions carrying more than one sync
    wait. Move extra waits onto same-engine NoOps inserted just before the
    instruction (same-engine program order preserves the semantics)."""
    n = 0
    uid = 0
    for f in nc.m.functions:
        for bb in f.blocks:
            il = bb.instructions
            new = []
            for inst in il:
                si = inst.sync_info
                waits = list(si.on_wait) if si and si.on_wait else []
                if len(waits) > max_waits:
                    for w in waits[:-max_waits]:
                        uid += 1
                        nop = mybir.InstNoOp(
                            name=f"I-wsplit-{uid}", ins=[], outs=[])
                        nop.engine = inst.engine
                        nop.sync_info = mybir.SyncInfo(
                            on_wait=[w], on_update=[])
                        new.append(nop)
                        n += 1
                    si.on_wait = waits[-max_waits:]
                new.append(inst)
            il[:] = new
    return n


def apply():
    tile.TileContext._drain_and_barrier = _patched_drain_and_barrier


F32 = mybir.dt.float32
F16 = mybir.dt.float16
F8 = mybir.dt.float8e4
I32 = mybir.dt.int32

B, T, L = 64, 50, 64
VOCAB, EMB = 30000, 300
NF = 256
FS = (3, 4, 5)
HID = 1024
OUT = 32
N_CORES = 8
B_LOC = B // N_CORES          # 8
UTT = B_LOC * T               # 400 utterances per core
GROUPS = T                    # one group = one timestep = 8 utterances
TOK_PER_GROUP = B_LOC * L     # 512
E_CHUNKS = [(0, 128), (128, 128), (256, 44)]
G4 = 4 * HID                  # 4096
QS = HID // 4                 # 256: gate quarter (one PE column-tile stream)
WSCALE = 16.0                 # fp8/fp16 weight pre-scale (1/16 on h via ident)


def build_nc(phases="all", n_groups=GROUPS, n_steps=T, no_gather=False):
    nc = bass.Bass()
    # ---- DRAM parameters (per-core inputs) ----
    tok = nc.declare_dram_parameter("tok", [UTT * L], I32, isOutput=False)
    emb = nc.declare_dram_parameter("emb", [VOCAB, EMB], F16, isOutput=False)
    w3 = nc.declare_dram_parameter("w3", [3, EMB, NF], F16, isOutput=False)
    w4 = nc.declare_dram_parameter("w4", [4, EMB, NF], F16, isOutput=False)
    w5 = nc.declare_dram_parameter("w5", [5, EMB, NF], F16, isOutput=False)
    cbias = nc.declare_dram_parameter("cbias", [3 * NF], F32, isOutput=False)
    h2oA = nc.declare_dram_parameter("h2oA", [768, OUT], F16, isOutput=False)
    h2oB = nc.declare_dram_parameter("h2oB", [HID, OUT], F16, isOutput=False)
    h2ob = nc.declare_dram_parameter("h2ob", [OUT], F32, isOutput=False)
    wih0b = nc.declare_dram_parameter("wih0b", [OUT + 2, G4], F16, isOutput=False)
    whh0T = nc.declare_dram_parameter("whh0T", [HID, G4], F16, isOutput=False)
    wih1T = nc.declare_dram_parameter("wih1T", [HID, G4], F8, isOutput=False)
    whh1T = nc.declare_dram_parameter("whh1T", [HID, G4], F8, isOutput=False)
    bseld = nc.declare_dram_parameter("bseld", [34, B_LOC], F16, isOutput=False)
    y = nc.declare_dram_parameter("y", [T, OUT, B_LOC], F32, isOutput=True)

    wdr = {3: w3, 4: w4, 5: w5}
    POS = {3: 62, 4: 61, 5: 60}
    # feats chunk index for (fs, ftile)
    CHUNK = {(3, 0): 0, (3, 1): 1, (4, 0): 2, (4, 1): 3, (5, 0): 4, (5, 1): 5}

    with tile.TileContext(nc) as tc:
        from contextlib import ExitStack

        with ExitStack() as root:
            const = root.enter_context(tc.tile_pool(name="const", bufs=1))
            persist = root.enter_context(tc.tile_pool(name="persist", bufs=1))

            # identity for PE transpose; identS = I/16 in fp16 (the diagonal
            # 8x8 sub-blocks at partitions 32m feed row-tiled transposes)
            ident = const.tile([128, 128], F32)
            make_identity(nc, ident[:])
            identS = const.tile([128, 128], F16)
            nc.vector.tensor_copy(out=identS[:], in_=ident[:])

            # resident LSTM weights + head
            wih0b_sb = persist.tile([OUT + 2, G4], F16, tag="wih0b")
            nc.sync.dma_start(out=wih0b_sb[:], in_=wih0b[:])
            whh0_sb = []
            for c in range(8):
                t_ = persist.tile([128, G4], F16, tag=f"whh0_{c}")
                nc.sync.dma_start(out=t_[:], in_=whh0T[c * 128:(c + 1) * 128, :])
                whh0_sb.append(t_)
            wih1_sb = []
            for c in range(8):
                t1_ = persist.tile([128, G4], F8, tag=f"wih1_{c}",
                                   name=f"wih1_{c}")
                nc.sync.dma_start(out=t1_[:], in_=wih1T[c * 128:(c + 1) * 128, :])
                wih1_sb.append(t1_)
            whh1_sb = []
            for c in range(8):
                t2_ = persist.tile([128, G4], F8, tag=f"whh1_{c}",
                                   name=f"whh1_{c}")
                nc.sync.dma_start(out=t2_[:], in_=whh1T[c * 128:(c + 1) * 128, :])
                whh1_sb.append(t2_)
            h2oB_sb = persist.tile([128, 8 * OUT], F16, tag="h2oB")
            for c in range(8):
                nc.sync.dma_start(
                    out=h2oB_sb[:, c * OUT:(c + 1) * OUT],
                    in_=h2oB[c * 128:(c + 1) * 128, :],
                )
            h2ob_sb = persist.tile([OUT, 1], F32, tag="h2ob")
            nc.sync.dma_start(out=h2ob_sb[:], in_=h2ob[:].rearrange("(a b) -> a b", b=1))

            # ftA (feats live only in the CNN stack)
            ftA_sb = persist.tile([OUT, UTT], F32, tag="ftA")

            # LSTM states
            prevT = persist.tile([OUT + 1, B_LOC], F16, tag="prevT")
            nc.vector.memset(prevT[:], 0.0)
            nc.vector.memset(prevT[OUT:OUT + 1, :], 1.0)
            h0T = persist.tile([128, 8 * B_LOC], F16, tag="h0T")
            h1T = persist.tile([128, 8 * B_LOC], F16, tag="h1T")
            nc.vector.memset(h0T[:], 0.0)
            nc.vector.memset(h1T[:], 0.0)
            # packed cell states: partition group 32m rows 0..7 = batch,
            # free col k = hid m*256+k
            c0 = persist.tile([104, QS], F32, tag="c0")
            c1 = persist.tile([104, QS], F32, tag="c1")
            nc.vector.memset(c0[:], 0.0)
            nc.vector.memset(c1[:], 0.0)
            bsel = persist.tile([34, B_LOC], F16, tag="bsel")
            nc.sync.dma_start(out=bsel[:], in_=bseld[:])

            # ---------------- CNN phase ----------------
            with ExitStack() as cnn:
                wpool = cnn.enter_context(tc.tile_pool(name="wconv", bufs=1))
                idxp = cnn.enter_context(tc.tile_pool(name="idx", bufs=1))
                gath = cnn.enter_context(tc.tile_pool(name="gath", bufs=12))
                xp = cnn.enter_context(tc.tile_pool(name="xp", bufs=2))
                relup = cnn.enter_context(tc.tile_pool(name="relup", bufs=3))
                ps_tr = cnn.enter_context(
                    tc.tile_pool(name="ps_tr", bufs=3, space="PSUM"))
                ps_conv = cnn.enter_context(
                    tc.tile_pool(name="ps_conv", bufs=4, space="PSUM"))

                # conv weights resident: per (fs, k, echunk) tile [ec, 256]
                wsb = {}
                for fs in FS:
                    for k in range(fs):
                        for ci, (e0, ec) in enumerate(E_CHUNKS):
                            t_ = wpool.tile([128, NF], F16, tag=f"w{fs}_{k}_{ci}")
                            nc.sync.dma_start(
                                out=t_[:ec, :], in_=wdr[fs][k, e0:e0 + ec, :])
                            wsb[(fs, k, ci)] = t_
                # conv biases [128, 6] (col = chunk)
                cb_sb = wpool.tile([128, 6], F32, tag="cb")
                for (fs, ft), ch in CHUNK.items():
                    off = {3: 0, 4: NF, 5: 2 * NF}[fs] + ft * 128
                    nc.sync.dma_start(
                        out=cb_sb[:, ch:ch + 1],
                        in_=cbias[off:off + 128].rearrange("(a b) -> a b", b=1))

                h2oA_sb = wpool.tile([128, 6 * OUT], F16, tag="h2oA")
                for c in range(6):
                    nc.sync.dma_start(
                        out=h2oA_sb[:, c * OUT:(c + 1) * OUT],
                        in_=h2oA[c * 128:(c + 1) * 128, :],
                    )
                feats_sb = wpool.tile([128, 6 * UTT], F16, tag="feats")
                # all token indices [128, 200]
                idx_sb = idxp.tile([128, UTT * L // 128], I32, tag="idx")
                nc.sync.dma_start(
                    out=idx_sb[:], in_=tok[:].rearrange("(g p) -> p g", p=128))

                for g in (range(n_groups) if phases in ("all", "cnn", "decoupled") else []):
                    # gather + transpose -> x_sb chunks [128, 512]
                    xs = [xp.tile([128, TOK_PER_GROUP], F16, tag=f"x{ci}",
                                  name=f"x{ci}")
                          for ci in range(3)]
                    for i in range(4):
                        rows = gath.tile([128, EMB], F32, tag="rows")
                        if no_gather:
                            nc.vector.memset(rows[:, :2], 0.0)
                        else:
                            nc.gpsimd.indirect_dma_start(
                                out=rows[:],
                                out_offset=None,
                                in_=emb[:],
                                in_offset=bass.IndirectOffsetOnAxis(
                                    ap=idx_sb[:, g * 4 + i:g * 4 + i + 1], axis=0),
                            )
                        for ci, (e0, ec) in enumerate(E_CHUNKS):
                            pt = ps_tr.tile([128, 128], F32, tag="pt")
                            nc.tensor.transpose(
                                out=pt[:ec, :], in_=rows[:, e0:e0 + ec],
                                identity=ident[:])
                            nc.vector.tensor_copy(
                                out=xs[ci][:ec, i * 128:(i + 1) * 128],
                                in_=pt[:ec, :])
                    # conv matmuls
                    for fs in FS:
                        npos = POS[fs]
                        for ft in range(2):
                            pc = ps_conv.tile([128, 8 * 62], F32, tag="conv")
                            first = True
                            for k in range(fs):
                                for ci, (e0, ec) in enumerate(E_CHUNKS):
                                    rhs = (xs[ci][:ec]
                                           .rearrange("e (n l) -> e n l", l=L)
                                           [:, :, k:k + npos])
                                    nc.tensor.matmul(
                                        pc[:, :8 * npos],
                                        lhsT=wsb[(fs, k, ci)][:ec,
                                             ft * 128:(ft + 1) * 128],
                                        rhs=rhs,
                                        start=first,
                                        stop=(k == fs - 1 and ci == 2),
                                    )
                                    first = False
                            ch = CHUNK[(fs, ft)]
                            relu = relup.tile([128, 8 * 62], F32, tag="relu")
                            nc.scalar.activation(
                                relu[:, :8 * npos], pc[:, :8 * npos],
                                mybir.ActivationFunctionType.Relu,
                                bias=cb_sb[:, ch:ch + 1])
                            nc.vector.tensor_reduce(
                                out=feats_sb[:, ch * UTT + g * 8:
                                             ch * UTT + (g + 1) * 8],
                                in_=relu[:, :8 * npos].rearrange(
                                    "f (n p) -> f n p", n=8),
                                axis=mybir.AxisListType.X,
                                op=mybir.AluOpType.max,
                            )
                # ftA = h2oA.T @ feats
                if phases == "lstm":
                    nc.vector.memset(feats_sb[:], 0.0)
                if phases == "decoupled":
                    nc.vector.memset(ftA_sb[:], 0.0)
                pf = ps_conv.tile([128, 8 * 62], F32, tag="conv")
                for c in (range(6) if phases != "decoupled" else []):
                    nc.tensor.matmul(
                        pf[:OUT, :UTT],
                        lhsT=h2oA_sb[:, c * OUT:(c + 1) * OUT],
                        rhs=feats_sb[:, c * UTT:(c + 1) * UTT],
                        start=(c == 0), stop=(c == 5),
                    )
                if phases != "decoupled":
                    nc.vector.tensor_copy(out=ftA_sb[:], in_=pf[:OUT, :UTT])

            # ---------------- LSTM phase ----------------
            # Gate layout: gate q (i,f,g,o), quarter m: PSUM/packed-SBUF
            # partitions 32m+0..7 (batch), free cols 0..255 = hid m*256..
            with ExitStack() as lst:
                gsb = lst.enter_context(tc.tile_pool(name="gates", bufs=1))
                tmpp = lst.enter_context(tc.tile_pool(name="tmp", bufs=1))
                outp = lst.enter_context(tc.tile_pool(name="outp", bufs=2))
                # 2 gate banks (gates run in 2 waves) + 4 transpose banks
                # (one per PE row group: concurrent row tiles must not share
                # a PSUM bank) + head bank
                ps_g = lst.enter_context(
                    tc.tile_pool(name="ps_g", bufs=3, space="PSUM"))
                ps_s = lst.enter_context(
                    tc.tile_pool(name="ps_s", bufs=1, space="PSUM"))

                ACTF = mybir.ActivationFunctionType
                gfun = [ACTF.Sigmoid, ACTF.Sigmoid, ACTF.Tanh, ACTF.Sigmoid]

                if phases == "cnn":
                    nc.vector.memset(ftA_sb[:], 0.0)

                def cell_update(gq, cstate, hname):
                    """c = sig_f*c + sig_i*tanh_g; h = sig_o*tanh(c).
                    All operands packed [104, 256]."""
                    t2 = tmpp.tile([104, QS], F16, tag="t2", name="t2")
                    nc.vector.tensor_mul(t2[:], gq[0][:], gq[2][:])
                    t1 = tmpp.tile([104, QS], F32, tag="t1", name="t1")
                    nc.vector.tensor_mul(t1[:], gq[1][:], cstate[:])
                    nc.vector.tensor_add(cstate[:], t1[:], t2[:])
                    th = tmpp.tile([104, QS], F16, tag="th", name="th")
                    nc.scalar.activation(th[:], cstate[:], ACTF.Tanh)
                    hP = tmpp.tile([104, QS], F16, tag="hP", name=hname)
                    nc.vector.tensor_mul(hP[:], gq[3][:], th[:])
                    return hP

                def h_transpose(hP, hT):
                    """packed h [104, 256] -> hT [128, 64] (chunk c at cols
                    c*8..c*8+8), scaled 1/16 by identS. One PSUM bank per PE
                    row group: concurrent row tiles must not share a bank."""
                    for m in range(4):
                        pt = ps_s.tile([128, 16], F16, tag=f"ptr{m}",
                                       name=f"pt{m}")
                        for h in range(2):
                            nc.tensor.transpose(
                                out=pt[:, h * 8:(h + 1) * 8],
                                in_=hP[32 * m:32 * m + 8,
                                       h * 128:(h + 1) * 128],
                                identity=identS[32 * m:32 * m + 8,
                                                32 * m:32 * m + 8],
                                tile_position=(32 * m, 0),
                            )
                        nc.vector.tensor_copy(
                            out=hT[:, 16 * m:16 * m + 16], in_=pt[:])

                def mm_rounds(ps, q, lhsT_tiles, w_tiles, start):
                    # one accumulation segment: 8 contraction chunks x 4
                    # concurrent column tiles
                    for c in range(8):
                        for m in range(4):
                            o = q * HID + m * QS
                            nc.tensor.matmul(
                                ps[32 * m:32 * m + 8, :],
                                lhsT=lhsT_tiles[:, c * 8:(c + 1) * 8],
                                rhs=w_tiles[c][:, o:o + QS],
                                start=(start and c == 0), stop=False,
                                tile_position=(0, 32 * m))

                def l1_finish(ps, q):
                    # wih1 rounds (need fresh h0T) + bias, closing the group
                    mm_rounds(ps, q, h0T, wih1_sb, start=False)
                    for m in range(4):
                        o = q * HID + m * QS
                        nc.tensor.matmul(
                            ps[32 * m:32 * m + 8, :],
                            lhsT=bsel[32:34, :],
                            rhs=wih0b_sb[32:34, o:o + QS],
                            start=False, stop=True,
                            tile_position=(32, 32 * m))

                def gate_act(ps, q, layer):
                    ga = gsb.tile([104, QS], F16, tag=f"g{layer}_{q}",
                                  name=f"g{layer}_{q}")
                    nc.scalar.activation(ga[:], ps[:], gfun[q],
                                         scale=1.0 / WSCALE)
                    return ga

                for t in (range(n_steps) if phases in ("all", "lstm", "decoupled") else []):
                    # ---- layer 0 gates: whh0 rounds first (prevT last, so
                    # the previous step's head hides under the stream) ----
                    g0 = []
                    for q in range(4):
                        ps = ps_g.tile([104, QS], F32, tag="psq",
                                       name=f"ps0_{q}")
                        mm_rounds(ps, q, h0T, whh0_sb, start=True)
                        for m in range(4):
                            o = q * HID + m * QS
                            nc.tensor.matmul(
                                ps[32 * m:32 * m + 8, :],
                                lhsT=prevT[:],
                                rhs=wih0b_sb[:OUT + 1, o:o + QS],
                                start=False, stop=True,
                                tile_position=(0, 32 * m))
                        g0.append(gate_act(ps, q, 0))
                    # ---- layer 0 cell update; l1 whh1 prestreams (which
                    # depend only on h1T of the previous step) hide it ----
                    hP0 = cell_update(g0, c0, "hP0")
                    ps1 = []
                    for q in range(3):
                        ps = ps_g.tile([104, QS], F32, tag="psq",
                                       name=f"ps1_{q}")
                        mm_rounds(ps, q, h1T, whh1_sb, start=True)
                        ps1.append(ps)
                    h_transpose(hP0, h0T)
                    # ---- layer 1 finish ----
                    g1 = []
                    for q in range(3):
                        l1_finish(ps1[q], q)
                        g1.append(gate_act(ps1[q], q, 1))
                    ps = ps_g.tile([104, QS], F32, tag="psq", name="ps1_3")
                    mm_rounds(ps, 3, h1T, whh1_sb, start=True)
                    l1_finish(ps, 3)
                    g1.append(gate_act(ps, 3, 1))
                    # ---- layer 1 cell update ----
                    hP1 = cell_update(g1, c1, "hP1")
                    h_transpose(hP1, h1T)

                    # ---- prediction head (h1T carries 1/16; h2oB is x16) ----
                    pp = ps_s.tile([OUT, B_LOC], F32, tag="ppred", name="pp")
                    for c in range(8):
                        nc.tensor.matmul(
                            pp[:], lhsT=h2oB_sb[:, c * OUT:(c + 1) * OUT],
                            rhs=h1T[:, c * 8:(c + 1) * 8],
                            start=(c == 0), stop=(c == 7))
                    pin = outp.tile([OUT, B_LOC], F32, tag="pin", name="pin")
                    nc.vector.tensor_add(pin[:], pp[:],
                                         ftA_sb[:, t * 8:(t + 1) * 8])
                    pred = outp.tile([OUT, B_LOC], F32, tag="pred", name="pred")
                    nc.scalar.activation(pred[:], pin[:], ACTF.Sigmoid,
                                         bias=h2ob_sb[:])
                    nc.vector.tensor_copy(out=prevT[:OUT, :], in_=pred[:])
                    nc.sync.dma_start(out=y[t], in_=pred[:])
    return nc


def prep_inputs(dialogue, embedding, cw3, cb3, cw4, cb4, cw5, cb5,
                wih0, whh0, b0, wih1, whh1, b1, h2o_w, h2o_b):
    """Host-side: shard + lay out per-core input maps."""
    import ml_dtypes
    f32 = np.float32
    f16 = np.float16
    f8 = ml_dtypes.float8_e4m3
    dial = np.asarray(dialogue).astype(np.int32)
    emb = np.ascontiguousarray(np.asarray(embedding, f32).astype(f16))
    w3p = np.ascontiguousarray(np.asarray(cw3, f32).transpose(2, 1, 0).astype(f16))
    w4p = np.ascontiguousarray(np.asarray(cw4, f32).transpose(2, 1, 0).astype(f16))
    w5p = np.ascontiguousarray(np.asarray(cw5, f32).transpose(2, 1, 0).astype(f16))
    cb = np.concatenate([np.asarray(cb3, f32), np.asarray(cb4, f32),
                         np.asarray(cb5, f32)])
    h2oA = np.ascontiguousarray(np.asarray(h2o_w, f32)[:, :768].T.astype(f16))
    h2oB = np.ascontiguousarray(np.asarray(h2o_w, f32)[:, 768:].T.astype(f16))
    h2ob = np.asarray(h2o_b, f32)
    wih0b = np.ascontiguousarray(
        (np.concatenate([np.asarray(wih0, f32).T,
                         np.asarray(b0, f32)[None, :],
                         np.asarray(b1, f32)[None, :]], 0) * WSCALE).astype(f16))
    whh0T = np.ascontiguousarray(
        (np.asarray(whh0, f32).T * WSCALE).astype(f16))
    wih1Tp = np.ascontiguousarray(
        (np.asarray(wih1, f32).T * WSCALE).astype(f8))
    whh1T = np.ascontiguousarray(
        (np.asarray(whh1, f32).T * WSCALE).astype(f8))

    bsel_np = np.zeros((34, B_LOC), f16)
    bsel_np[33, :] = 1.0
    in_maps = []
    for c in range(N_CORES):
        tok = np.ascontiguousarray(
            dial[c * B_LOC:(c + 1) * B_LOC].transpose(1, 0, 2).reshape(-1))
        in_maps.append({
            "tok": tok, "emb": emb, "w3": w3p, "w4": w4p, "w5": w5p,
            "cbias": cb, "h2oA": h2oA, "h2oB": h2oB, "h2ob": h2ob,
            "wih0b": wih0b, "whh0T": whh0T, "wih1T": wih1Tp, "whh1T": whh1T,
            "bseld": bsel_np,
        })
    return in_maps


def assemble_output(results):
    """results: list of 8 dicts with y [T, OUT, B_LOC] -> [B, T, OUT]."""
    outs = []
    for c in range(N_CORES):
        yc = results[c]["y"]                       # [50, 32, 8]
        outs.append(np.ascontiguousarray(yc.transpose(2, 0, 1)))
    return np.concatenate(outs, 0).astype(np.float32)


_CACHE = {}


def kernel(**inputs) -> np.ndarray:
    apply()  # tile workarounds (idempotent)
    if "nc" not in _CACHE:
        nc = build_nc()
        split_multiwait(nc)
        _CACHE["nc"] = nc
    nc = _CACHE["nc"]
    in_maps = prep_inputs(**inputs)
    last_err = None
    for _ in range(3):  # the axon execute path sporadically drops a run
        try:
            res = run_bass_kernel_spmd(nc, in_maps, core_ids=list(range(N_CORES)))
            return assemble_output(res.results)
        except Exception as e:  # noqa: BLE001 - retry transient runtime faults
            last_err = e
    raise last_err


# revision 24
# speedup vs baseline: 5.8280x; 5.8280x over previous
"""Trainium2 Bass kernel for nn_DAMIC_88235808129614.

TextCNN (embed -> conv fs=3/4/5 -> relu -> maxpool) + 2-layer LSTM + sigmoid
head. Data-parallel over batch across 8 NeuronCores; the sequential timestep
loop runs locally per shard (no collectives).

LSTM gate matmuls use 4x PE column tiling (128x32 mode): the batch dim is
only 8, so 4 independent gate-quarter streams share the array, each writing
its own PSUM partition group (one gate = 4 concurrent 256-col weight
streams). All recurrent weights are SBUF-resident: whh0 in fp16, wih1/whh1
in fp8e4 (all gate-weight inputs pre-scaled x16 so fp8 values sit in the
e4m3 normal range; the gate activations apply scale=1/16). Per step, the
weight streams that depend only on last-step state (whh0, whh1) issue
before those needing fresh state (prevT, wih1), hiding the cell-update and
head latency under PE streaming. h transposes are row-tiled PE transposes,
one PSUM bank per row group (concurrent row tiles must not share a bank).
The CNN embeds via a casting indirect gather (f16 table -> f32 rows) and
per-128-token PE transposes; conv runs as E-chunked matmuls with
relu+maxpool fused on ACT/DVE.

kernel(**inputs) takes the FULL unsharded inputs and returns [64, 50, 32] f32.
"""
import numpy as np

import concourse.bass as bass
import concourse.mybir as mybir
import concourse.tile as tile
from concourse.bass_utils import run_bass_kernel_spmd
from concourse.masks import make_identity


def _patched_drain_and_barrier(self, tick_clock, wait_clock):
    drain_inst = self.nc.sync.drain()
    wait_clock.add_sem_waits(
        drain_inst.ins, tile.ScopedClock({None: tick_clock.global_clock})
    )
    si = drain_inst.ins.sync_info
    waits = list(si.on_wait) if si and si.on_wait else []
    if len(waits) > 1:
        si.on_wait = waits[:1]
        for w in waits[1:]:
            nop = self.nc.sync.nop(nofuse=True, hint="split_drain_wait")
            nsi = nop.ins.sync_info
            if nsi is None:
                nop.ins.sync_info = mybir.SyncInfo(on_wait=[w], on_update=[])
            else:
                nsi.on_wait = [w]
    self.nc.all_engine_barrier()
    assert self.sems is not None
    popped = self.nc._tile_sem_poison_stack.pop()
    assert popped is self._sem_poison
    self.nc.clear_and_free_semaphores(list(self.sems.allocated().values()))
    self.nc.all_engine_barrier()


def split_multiwait(nc, max_waits=1):
    """This walrus build rejects instructions carrying more than one sync
    wait. Move extra waits onto same-engine NoOps inserted just before the
    instruction (same-engine program order preserves the semantics)."""
    n = 0
    uid = 0
    for f in nc.m.functions:
        for bb in f.blocks:
            il = bb.instructions
            new = []
            for inst in il:
                si = inst.sync_info
                waits = list(si.on_wait) if si and si.on_wait else []
                if len(waits) > max_waits:
                    for w in waits[:-max_waits]:
                        uid += 1
                        nop = mybir.InstNoOp(
                            name=f"I-wsplit-{uid}", ins=[], outs=[])
                        nop.engine = inst.engine
                        nop.sync_info = mybir.SyncInfo(
                            on_wait=[w], on_update=[])
                        new.append(nop)
                        n += 1
                    si.on_wait = waits[-max_waits:]
                new.append(inst)
            il[:] = new
    return n


def apply():
    tile.TileContext._drain_and_barrier = _patched_drain_and_barrier


F32 = mybir.dt.float32
F16 = mybir.dt.float16
F8 = mybir.dt.float8e4
I32 = mybir.dt.int32

B, T, L = 64, 50, 64
VOCAB, EMB = 30000, 300
NF = 256
FS = (3, 4, 5)
HID = 1024
OUT = 32
N_CORES = 8
B_LOC = B // N_CORES          # 8
UTT = B_LOC * T               # 400 utterances per core
GROUPS = T                    # one group = one timestep = 8 utterances
TOK_PER_GROUP = B_LOC * L     # 512
E_CHUNKS = [(0, 128), (128, 128), (256, 44)]
G4 = 4 * HID                  # 4096
QS = HID // 4                 # 256: gate quarter (one PE column-tile stream)
WSCALE = 16.0                 # fp8/fp16 weight pre-scale (1/16 on h via ident)


def build_nc(phases="all", n_groups=GROUPS, n_steps=T, no_gather=False):
    nc = bass.Bass()
    # ---- DRAM parameters (per-core inputs) ----
    tok = nc.declare_dram_parameter("tok", [UTT * L], I32, isOutput=False)
    emb = nc.declare_dram_parameter("emb", [VOCAB, EMB], F16, isOutput=False)
    w3 = nc.declare_dram_parameter("w3", [3, EMB, NF], F16, isOutput=False)
    w4 = nc.declare_dram_parameter("w4", [4, EMB, NF], F16, isOutput=False)
    w5 = nc.declare_dram_parameter("w5", [5, EMB, NF], F16, isOutput=False)
    cbias = nc.declare_dram_parameter("cbias", [3 * NF], F32, isOutput=False)
    h2oA = nc.declare_dram_parameter("h2oA", [768, OUT], F16, isOutput=False)
    h2oB = nc.declare_dram_parameter("h2oB", [HID, OUT], F16, isOutput=False)
    h2ob = nc.declare_dram_parameter("h2ob", [OUT], F32, isOutput=False)
    wih0b = nc.declare_dram_parameter("wih0b", [OUT + 2, G4], F16, isOutput=False)
    whh0T = nc.declare_dram_parameter("whh0T", [HID, G4], F16, isOutput=False)
    wih1T = nc.declare_dram_parameter("wih1T", [HID, G4], F8, isOutput=False)
    whh1T = nc.declare_dram_parameter("whh1T", [HID, G4], F8, isOutput=False)
    bseld = nc.declare_dram_parameter("bseld", [34, B_LOC], F16, isOutput=False)
    y = nc.declare_dram_parameter("y", [T, OUT, B_LOC], F32, isOutput=True)

    wdr = {3: w3, 4: w4, 5: w5}
    POS = {3: 62, 4: 61, 5: 60}
    # feats chunk index for (fs, ftile)
    CHUNK = {(3, 0): 0, (3, 1): 1, (4, 0): 2, (4, 1): 3, (5, 0): 4, (5, 1): 5}

    with tile.TileContext(nc) as tc:
        from contextlib import ExitStack

        with ExitStack() as root:
            const = root.enter_context(tc.tile_pool(name="const", bufs=1))
            persist = root.enter_context(tc.tile_pool(name="persist", bufs=1))

            # identity for PE transpose; identS = I/16 in fp16 (the diagonal
            # 8x8 sub-blocks at partitions 32m feed row-tiled transposes)
            ident = const.tile([128, 128], F32)
            make_identity(nc, ident[:])
            identS = const.tile([128, 128], F16)
            nc.vector.tensor_copy(out=identS[:], in_=ident[:])

            # resident LSTM weights + head
            wih0b_sb = persist.tile([OUT + 2, G4], F16, tag="wih0b")
            nc.sync.dma_start(out=wih0b_sb[:], in_=wih0b[:])
            whh0_sb = []
            for c in range(8):
                t_ = persist.tile([128, G4], F16, tag=f"whh0_{c}")
                nc.sync.dma_start(out=t_[:], in_=whh0T[c * 128:(c + 1) * 128, :])
                whh0_sb.append(t_)
            wih1_sb = []
            for c in range(8):
                t1_ = persist.tile([128, G4], F8, tag=f"wih1_{c}",
                                   name=f"wih1_{c}")
                nc.sync.dma_start(out=t1_[:], in_=wih1T[c * 128:(c + 1) * 128, :])
                wih1_sb.append(t1_)
            whh1_sb = []
            for c in range(8):
                t2_ = persist.tile([128, G4], F8, tag=f"whh1_{c}",
                                   name=f"whh1_{c}")
                nc.sync.dma_start(out=t2_[:], in_=whh1T[c * 128:(c + 1) * 128, :])
                whh1_sb.append(t2_)
            h2oB_sb = persist.tile([128, 8 * OUT], F16, tag="h2oB")
            for c in range(8):
                nc.sync.dma_start(
                    out=h2oB_sb[:, c * OUT:(c + 1) * OUT],
                    in_=h2oB[c * 128:(c + 1) * 128, :],
                )
            h2ob_sb = persist.tile([OUT, 1], F32, tag="h2ob")
            nc.sync.dma_start(out=h2ob_sb[:], in_=h2ob[:].rearrange("(a b) -> a b", b=1))

            # ftA (feats live only in the CNN stack)
            ftA_sb = persist.tile([OUT, UTT], F32, tag="ftA")

            # LSTM states
            prevT = persist.tile([OUT + 1, B_LOC], F16, tag="prevT")
            nc.vector.memset(prevT[:], 0.0)
            nc.vector.memset(prevT[OUT:OUT + 1, :], 1.0)
            h0T = persist.tile([128, 8 * B_LOC], F16, tag="h0T")
            h1T = persist.tile([128, 8 * B_LOC], F16, tag="h1T")
            nc.vector.memset(h0T[:], 0.0)
            nc.vector.memset(h1T[:], 0.0)
            # packed cell states: partition group 32m rows 0..7 = batch,
            # free col k = hid m*256+k
            c0 = persist.tile([104, QS], F32, tag="c0")
            c1 = persist.tile([104, QS], F32, tag="c1")
            nc.vector.memset(c0[:], 0.0)
            nc.vector.memset(c1[:], 0.0)
            bsel = persist.tile([34, B_LOC], F16, tag="bsel")
            nc.sync.dma_start(out=bsel[:], in_=bseld[:])

            # ---------------- CNN phase ----------------
            with ExitStack() as cnn:
                wpool = cnn.enter_context(tc.tile_pool(name="wconv", bufs=1))
                idxp = cnn.enter_context(tc.tile_pool(name="idx", bufs=1))
                gath = cnn.enter_context(tc.tile_pool(name="gath", bufs=12))
                xp = cnn.enter_context(tc.tile_pool(name="xp", bufs=2))
                relup = cnn.enter_context(tc.tile_pool(name="relup", bufs=3))
                ps_tr = cnn.enter_context(
                    tc.tile_pool(name="ps_tr", bufs=3, space="PSUM"))
                ps_conv = cnn.enter_context(
                    tc.tile_pool(name="ps_conv", bufs=4, space="PSUM"))

                # conv weights resident: per (fs, k, echunk) tile [ec, 256]
                wsb = {}
                for fs in FS:
                    for k in range(fs):
                        for ci, (e0, ec) in enumerate(E_CHUNKS):
                            t_ = wpool.tile([128, NF], F16, tag=f"w{fs}_{k}_{ci}")
                            nc.sync.dma_start(
                                out=t_[:ec, :], in_=wdr[fs][k, e0:e0 + ec, :])
                            wsb[(fs, k, ci)] = t_
                # conv biases [128, 6] (col = chunk)
                cb_sb = wpool.tile([128, 6], F32, tag="cb")
                for (fs, ft), ch in CHUNK.items():
                    off = {3: 0, 4: NF, 5: 2 * NF}[fs] + ft * 128
                    nc.sync.dma_start(
                        out=cb_sb[:, ch:ch + 1],
                        in_=cbias[off:off + 128].rearrange("(a b) -> a b", b=1))

                h2oA_sb = wpool.tile([128, 6 * OUT], F16, tag="h2oA")
                for c in range(6):
                    nc.sync.dma_start(
                        out=h2oA_sb[:, c * OUT:(c + 1) * OUT],
                        in_=h2oA[c * 128:(c + 1) * 128, :],
                    )
                feats_sb = wpool.tile([128, 6 * UTT], F16, tag="feats")
                # all token indices [128, 200]
                idx_sb = idxp.tile([128, UTT * L // 128], I32, tag="idx")
                nc.sync.dma_start(
                    out=idx_sb[:], in_=tok[:].rearrange("(g p) -> p g", p=128))

                for g in (range(n_groups) if phases in ("all", "cnn", "decoupled") else []):
                    # gather + transpose -> x_sb chunks [128, 512]
                    xs = [xp.tile([128, TOK_PER_GROUP], F16, tag=f"x{ci}",
                                  name=f"x{ci}")
                          for ci in range(3)]
                    for i in range(4):
                        rows = gath.tile([128, EMB], F32, tag="rows")
                        if no_gather:
                            nc.vector.memset(rows[:, :2], 0.0)
                        else:
                            nc.gpsimd.indirect_dma_start(
                                out=rows[:],
                                out_offset=None,
                                in_=emb[:],
                                in_offset=bass.IndirectOffsetOnAxis(
                                    ap=idx_sb[:, g * 4 + i:g * 4 + i + 1], axis=0),
                            )
                        for ci, (e0, ec) in enumerate(E_CHUNKS):
                            pt = ps_tr.tile([128, 128], F32, tag="pt")
                            nc.tensor.transpose(
                                out=pt[:ec, :], in_=rows[:, e0:e0 + ec],
                                identity=ident[:])
                            nc.vector.tensor_copy(
                                out=xs[ci][:ec, i * 128:(i + 1) * 128],
                                in_=pt[:ec, :])
                    # conv matmuls
                    for fs in FS:
                        npos = POS[fs]
                        for ft in range(2):
                            pc = ps_conv.tile([128, 8 * 62], F32, tag="conv")
                            first = True
                            for k in range(fs):
                                for ci, (e0, ec) in enumerate(E_CHUNKS):
                                    rhs = (xs[ci][:ec]
                                           .rearrange("e (n l) -> e n l", l=L)
                                           [:, :, k:k + npos])
                                    nc.tensor.matmul(
                                        pc[:, :8 * npos],
                                        lhsT=wsb[(fs, k, ci)][:ec,
                                             ft * 128:(ft + 1) * 128],
                                        rhs=rhs,
                                        start=first,
                                        stop=(k == fs - 1 and ci == 2),
                                    )
                                    first = False
                            ch = CHUNK[(fs, ft)]
                            relu = relup.tile([128, 8 * 62], F32, tag="relu")
                            nc.scalar.activation(
                                relu[:, :8 * npos], pc[:, :8 * npos],
                                mybir.ActivationFunctionType.Relu,
                                bias=cb_sb[:, ch:ch + 1])
                            nc.vector.tensor_reduce(
                                out=feats_sb[:, ch * UTT + g * 8:
                                             ch * UTT + (g + 1) * 8],
                                in_=relu[:, :8 * npos].rearrange(
                                    "f (n p) -> f n p", n=8),
                                axis=mybir.AxisListType.X,
                                op=mybir.AluOpType.max,
                            )
                # ftA = h2oA.T @ feats
                if phases == "lstm":
                    nc.vector.memset(feats_sb[:], 0.0)
                if phases == "decoupled":
                    nc.vector.memset(ftA_sb[:], 0.0)
                pf = ps_conv.tile([128, 8 * 62], F32, tag="conv")
                for c in (range(6) if phases != "decoupled" else []):
                    nc.tensor.matmul(
                        pf[:OUT, :UTT],
                        lhsT=h2oA_sb[:, c * OUT:(c + 1) * OUT],
                        rhs=feats_sb[:, c * UTT:(c + 1) * UTT],
                        start=(c == 0), stop=(c == 5),
                    )
                if phases != "decoupled":
                    nc.vector.tensor_copy(out=ftA_sb[:], in_=pf[:OUT, :UTT])

            # ---------------- LSTM phase ----------------
            # Gate layout: gate q (i,f,g,o), quarter m: PSUM/packed-SBUF
            # partitions 32m+0..7 (batch), free cols 0..255 = hid m*256..
            with ExitStack() as lst:
                gsb = lst.enter_context(tc.tile_pool(name="gates", bufs=1))
                tmpp = lst.enter_context(tc.tile_pool(name="tmp", bufs=1))
                outp = lst.enter_context(tc.tile_pool(name="outp", bufs=2))
                # 2 gate banks (gates run in 2 waves) + 4 transpose banks
                # (one per PE row group: concurrent row tiles must not share
                # a PSUM bank) + head bank
                ps_g = lst.enter_context(
                    tc.tile_pool(name="ps_g", bufs=3, space="PSUM"))
                ps_s = lst.enter_context(
                    tc.tile_pool(name="ps_s", bufs=1, space="PSUM"))

                ACTF = mybir.ActivationFunctionType
                gfun = [ACTF.Sigmoid, ACTF.Sigmoid, ACTF.Tanh, ACTF.Sigmoid]

                if phases == "cnn":
                    nc.vector.memset(ftA_sb[:], 0.0)

                def cell_update(gq, cstate, hname):
                    """c = sig_f*c + sig_i*tanh_g; h = sig_o*tanh(c).
                    All operands packed [104, 256]."""
                    t2 = tmpp.tile([104, QS], F16, tag="t2", name="t2")
                    nc.vector.tensor_mul(t2[:], gq[0][:], gq[2][:])
                    t1 = tmpp.tile([104, QS], F32, tag="t1", name="t1")
                    nc.vector.tensor_mul(t1[:], gq[1][:], cstate[:])
                    nc.vector.tensor_add(cstate[:], t1[:], t2[:])
                    th = tmpp.tile([104, QS], F16, tag="th", name="th")
                    nc.scalar.activation(th[:], cstate[:], ACTF.Tanh)
                    hP = tmpp.tile([104, QS], F16, tag="hP", name=hname)
                    nc.vector.tensor_mul(hP[:], gq[3][:], th[:])
                    return hP

                def h_transpose(hP, hT):
                    """packed h [104, 256] -> hT [128, 64] (chunk c at cols
                    c*8..c*8+8), scaled 1/16 by identS. One PSUM bank per PE
                    row group: concurrent row tiles must not share a bank."""
                    for m in range(4):
                        pt = ps_s.tile([128, 16], F16, tag=f"ptr{m}",
                                       name=f"pt{m}")
                        for h in range(2):
                            nc.tensor.transpose(
                                out=pt[:, h * 8:(h + 1) * 8],
                                in_=hP[32 * m:32 * m + 8,
                                       h * 128:(h + 1) * 128],
                                identity=identS[32 * m:32 * m + 8,
                                                32 * m:32 * m + 8],
                                tile_position=(32 * m, 0),
                            )
                        nc.vector.tensor_copy(
                            out=hT[:, 16 * m:16 * m + 16], in_=pt[:])

                def mm_rounds(ps, q, lhsT_tiles, w_tiles, start):
                    # one accumulation segment: 8 contraction chunks x 4
                    # concurrent column tiles
                    for c in range(8):
                        for m in range(4):
                            o = q * HID + m * QS
                            nc.tensor.matmul(
                                ps[32 * m:32 * m + 8, :],
                                lhsT=lhsT_tiles[:, c * 8:(c + 1) * 8],
                                rhs=w_tiles[c][:, o:o + QS],
                                start=(start and c == 0), stop=False,
                                tile_position=(0, 32 * m))

                def l1_finish(ps, q):
                    # wih1 rounds (need fresh h0T) + bias, closing the group
                    mm_rounds(ps, q, h0T, wih1_sb, start=False)
                    for m in range(4):
                        o = q * HID + m * QS
                        nc.tensor.matmul(
                            ps[32 * m:32 * m + 8, :],
                            lhsT=bsel[32:34, :],
                            rhs=wih0b_sb[32:34, o:o + QS],
                            start=False, stop=True,
                            tile_position=(32, 32 * m))

                def gate_act(ps, q, layer):
                    ga = gsb.tile([104, QS], F16, tag=f"g{layer}_{q}",
                                  name=f"g{layer}_{q}")
                    nc.scalar.activation(ga[:], ps[:], gfun[q],
                                         scale=1.0 / WSCALE)
                    return ga

                pre0 = None
                for t in (range(n_steps) if phases in ("all", "lstm", "decoupled") else []):
                    # ---- layer 0 gates: whh0 rounds first (prevT last, so
                    # the previous step's head hides under the stream).
                    # Gates 0..2 may arrive prestreamed from the previous
                    # step (whh0 rounds already issued against h0(t-1)). ----
                    g0 = []
                    for q in range(4):
                        if pre0 is not None and q < 3:
                            ps = pre0[q]
                        else:
                            ps = ps_g.tile([104, QS], F32, tag="psq",
                                           name=f"ps0_{q}")
                            mm_rounds(ps, q, h0T, whh0_sb, start=True)
                        for m in range(4):
                            o = q * HID + m * QS
                            nc.tensor.matmul(
                                ps[32 * m:32 * m + 8, :],
                                lhsT=prevT[:],
                                rhs=wih0b_sb[:OUT + 1, o:o + QS],
                                start=False, stop=True,
                                tile_position=(0, 32 * m))
                        g0.append(gate_act(ps, q, 0))
                    # ---- layer 0 cell update; l1 whh1 prestreams (which
                    # depend only on h1T of the previous step) hide it ----
                    hP0 = cell_update(g0, c0, "hP0")
                    ps1 = []
                    for q in range(3):
                        ps = ps_g.tile([104, QS], F32, tag="psq",
                                       name=f"ps1_{q}")
                        mm_rounds(ps, q, h1T, whh1_sb, start=True)
                        ps1.append(ps)
                    h_transpose(hP0, h0T)
                    # ---- layer 1 finish ----
                    g1 = []
                    for q in range(3):
                        l1_finish(ps1[q], q)
                        g1.append(gate_act(ps1[q], q, 1))
                    ps = ps_g.tile([104, QS], F32, tag="psq", name="ps1_3")
                    mm_rounds(ps, 3, h1T, whh1_sb, start=True)
                    l1_finish(ps, 3)
                    g1.append(gate_act(ps, 3, 1))
                    # ---- prestream next step's l0 whh0 rounds (h0(t) is
                    # final); hides the l1 cell + transposes + head chain ----
                    if t < n_steps - 1:
                        pre0 = []
                        for q in range(3):
                            ps = ps_g.tile([104, QS], F32, tag="psq",
                                           name=f"pre0_{q}")
                            mm_rounds(ps, q, h0T, whh0_sb, start=True)
                            pre0.append(ps)
                    else:
                        pre0 = None
                    # ---- layer 1 cell update ----
                    hP1 = cell_update(g1, c1, "hP1")
                    h_transpose(hP1, h1T)

                    # ---- prediction head (h1T carries 1/16; h2oB is x16) ----
                    pp = ps_s.tile([OUT, B_LOC], F32, tag="ppred", name="pp")
                    for c in range(8):
                        nc.tensor.matmul(
                            pp[:], lhsT=h2oB_sb[:, c * OUT:(c + 1) * OUT],
                            rhs=h1T[:, c * 8:(c + 1) * 8],
                            start=(c == 0), stop=(c == 7))
                    pin = outp.tile([OUT, B_LOC], F32, tag="pin", name="pin")
                    nc.vector.tensor_add(pin[:], pp[:],
                                         ftA_sb[:, t * 8:(t + 1) * 8])
                    pred = outp.tile([OUT, B_LOC], F32, tag="pred", name="pred")
                    nc.scalar.activation(pred[:], pin[:], ACTF.Sigmoid,
                                         bias=h2ob_sb[:])
                    nc.vector.tensor_copy(out=prevT[:OUT, :], in_=pred[:])
                    nc.sync.dma_start(out=y[t], in_=pred[:])
    return nc


def prep_inputs(dialogue, embedding, cw3, cb3, cw4, cb4, cw5, cb5,
                wih0, whh0, b0, wih1, whh1, b1, h2o_w, h2o_b):
    """Host-side: shard + lay out per-core input maps."""
    import ml_dtypes
    f32 = np.float32
    f16 = np.float16
    f8 = ml_dtypes.float8_e4m3
    dial = np.asarray(dialogue).astype(np.int32)
    emb = np.ascontiguousarray(np.asarray(embedding, f32).astype(f16))
    w3p = np.ascontiguousarray(np.asarray(cw3, f32).transpose(2, 1, 0).astype(f16))
    w4p = np.ascontiguousarray(np.asarray(cw4, f32).transpose(2, 1, 0).astype(f16))
    w5p = np.ascontiguousarray(np.asarray(cw5, f32).transpose(2, 1, 0).astype(f16))
    cb = np.concatenate([np.asarray(cb3, f32), np.asarray(cb4, f32),
                         np.asarray(cb5, f32)])
    h2oA = np.ascontiguousarray(np.asarray(h2o_w, f32)[:, :768].T.astype(f16))
    h2oB = np.ascontiguousarray(np.asarray(h2o_w, f32)[:, 768:].T.astype(f16))
    h2ob = np.asarray(h2o_b, f32)
    wih0b = np.ascontiguousarray(
        (np.concatenate([np.asarray(wih0, f32).T,
                         np.asarray(b0, f32)[None, :],
                         np.asarray(b1, f32)[None, :]], 0) * WSCALE).astype(f16))
    whh0T = np.ascontiguousarray(
        (np.asarray(whh0, f32).T * WSCALE).astype(f16))
    wih1Tp = np.ascontiguousarray(
        (np.asarray(wih1, f32).T * WSCALE).astype(f8))
    whh1T = np.ascontiguousarray(
        (np.asarray(whh1, f32).T * WSCALE).astype(f8))

    bsel_np = np.zeros((34, B_LOC), f16)
    bsel_np[33, :] = 1.0
    in_maps = []
    for c in range(N_CORES):
        tok = np.ascontiguousarray(
            dial[c * B_LOC:(c + 1) * B_LOC].transpose(1, 0, 2).reshape(-1))
        in_maps.append({
            "tok": tok, "emb": emb, "w3": w3p, "w4": w4p, "w5": w5p,
            "cbias": cb, "h2oA": h2oA, "h2oB": h2oB, "h2ob": h2ob,
            "wih0b": wih0b, "whh0T": whh0T, "wih1T": wih1Tp, "whh1T": whh1T,
            "bseld": bsel_np,
        })
    return in_maps


def assemble_output(results):
    """results: list of 8 dicts with y [T, OUT, B_LOC] -> [B, T, OUT]."""
    outs = []
    for c in range(N_CORES):
        yc = results[c]["y"]                       # [50, 32, 8]
        outs.append(np.ascontiguousarray(yc.transpose(2, 0, 1)))
    return np.concatenate(outs, 0).astype(np.float32)


_CACHE = {}


def kernel(**inputs) -> np.ndarray:
    apply()  # tile workarounds (idempotent)
    if "nc" not in _CACHE:
        nc = build_nc()
        split_multiwait(nc)
        _CACHE["nc"] = nc
    nc = _CACHE["nc"]
    in_maps = prep_inputs(**inputs)
    last_err = None
    for _ in range(3):  # the axon execute path sporadically drops a run
        try:
            res = run_bass_kernel_spmd(nc, in_maps, core_ids=list(range(N_CORES)))
            return assemble_output(res.results)
        except Exception as e:  # noqa: BLE001 - retry transient runtime faults
            last_err = e
    raise last_err
